# revision 37
# baseline (speedup 1.0000x reference)
"""Trainium2 Bass kernel for a ReActNet binary BasicBlock.

Reference computation (per reference.py):
    a   = sign(x)                              # forward of BinaryActivation
    bw  = alpha * sign(w), alpha = mean|w| over (in,kh,kw) per out-channel
    y   = conv3x3(a, bw, stride 1, pad 1)      # NCHW
    out = BN_train(y) * gamma + beta + x       # batch stats over (N,H,W)

Key identities:
  * y = alpha_k * z with z = conv3x3(sign(x), sign(w)) an exact small-integer
    tensor, so the conv runs on the PE array in fp8 DoubleRow mode (+-1 is
    exact in fp8e4) with exact fp32 accumulation.
  * BN(y)*gamma+beta = z*scale_k + bias_k with
        scale_k = gamma_k * alpha_k / sqrt(alpha_k^2 * var_z,k + eps)
        bias_k  = beta_k - mu_z,k * scale_k
    where mu_z/var_z are global batch stats of z. Per-core partial sums are
    exchanged with one small AllGather per 128-channel group; each core
    reduces the 8 gathered partials locally.

Sharding: data-parallel over batch, 4 images per core on 8 cores.

Schedule (per core):
  * x arrives as 16 half-image DMAs; ACT signs each half into the padded
    fp8 buffer, DVE casts it to f16 for the residual (2x copy mode; the
    gpsimd cast measured ~6x slower on HW and back-pressured the ring).
  * a_s is laid out [n][row][cg][64-col] so a conv tile's read interval
    under Tile's interval dependency tracking spans only its own rows —
    just-in-time sign emission then never creates false cross-image or
    cross-half dependencies, and the x stream, signs, and conv tiles
    pipeline at DMA pace.
  * conv tiles run kg0-first (kg1 rows 0-2 interleave as PE load-phase
    work, rows 3-6 follow kg0), so kg0's stats AllGather launches right
    after the last x half and hides under the kg1 leftovers; kg1's
    AllGather queues behind it on the collective cores while kg0's
    output stores stream.
  * per-kg stats scalars live in separate tiles and separate DRAM pools
    (shared tiles/pools picked up false cross-kg deps that serialized
    the two collectives), and kg1's post-gather math is interleaved into
    pass2(0) in <=3-op groups bypassed via the 4-deep engine wait queues.
  * PSUM evacuations split across ACT/DVE per-phase to keep both under
    the x-DMA pace; pass2 streams eighth/quarter-sized leading chunks so
    the first store launches early, then half-image chunks gated only by
    the store DMA.
"""

import numpy as np

try:
    import concourse.bass as bass
except ImportError:  # pragma: no cover
    import sys

    for p in ("/opt/trn_rl_repo", "/root/.axon_site/_ro/trn_rl_repo"):
        sys.path.insert(0, p)
    import concourse.bass as bass

import concourse.tile as tile
from concourse import bacc, bass_utils, mybir
from concourse.masks import make_identity

F32 = mybir.dt.float32
F16 = mybir.dt.float16
F8 = mybir.dt.float8e4

N, C, H, W = 32, 256, 56, 56
NCORES = 8
NLOC = N // NCORES  # images per core
HP, WP = H + 2, W + 2  # zero-padded image
HW = H * W
HR = H // 2  # rows per half-image load
HH = HR * W  # elements per half image
WP2 = 64  # row pitch: 58 cols padded to /16 (DoubleRow stride rule)
RT = 8  # padded rows per PSUM tile
NRT = H // RT  # row tiles per image
FT = RT * WP  # matmul free size (464, incl. 2 pad columns per row)
CG = C // 128  # channel groups of 128
EPS = 1e-5
M_TOTAL = float(N * H * W)  # BN element count per channel
M_LOCAL = float(NLOC * H * W)
W_RED = float(C * 9)  # alpha divisor


def _build_kernel():
    nc = bacc.Bacc(
        "TRN2", target_bir_lowering=False, debug=False, num_devices=NCORES
    )
    x_d = nc.dram_tensor("x", (NLOC, C, H, W), F32, kind="ExternalInput").ap()
    w_d = nc.dram_tensor("weights", (C, C, 3, 3), F32, kind="ExternalInput").ap()
    g_d = nc.dram_tensor("gamma", (C,), F32, kind="ExternalInput").ap()
    b_d = nc.dram_tensor("beta", (C,), F32, kind="ExternalInput").ap()
    o_d = nc.dram_tensor("out", (NLOC, C, H, W), F32, kind="ExternalOutput").ap()

    with tile.TileContext(nc) as tc:
        with (
            tc.tile_pool(name="consts", bufs=1) as consts,
            tc.tile_pool(name="persist", bufs=1) as persist,
            tc.tile_pool(name="xstage", bufs=4) as xstage,
            tc.tile_pool(name="psum", bufs=6, space="PSUM") as psum_pool,
            tc.tile_pool(name="psum_t", bufs=2, space="PSUM") as psum_t,
            tc.tile_pool(name="dram0", bufs=1, space="DRAM") as dram0,
            tc.tile_pool(name="dram1", bufs=1, space="DRAM") as dram1,
        ):
            # ---- persistent SBUF state ----
            # padded sign(x): [n][row][cg][col].  Image outermost and cg
            # interleaved INSIDE the row dim, so a conv tile's read interval
            # spans only rows rt..rt+9 of its own image — h1 signs never
            # falsely conflict with h0 tiles under interval dep tracking.
            a_s = persist.tile([128, NLOC, HP, CG, WP2], F8)
            x16 = persist.tile([128, CG, NLOC, HW], F16)  # x for residual
            z16 = persist.tile([128, CG, NLOC, HW], F16)  # conv output
            # sign(w): [c, kg, off, cg, k].  kg outermost so kg0 reads never
            # alias kg1 writes; off next so each Ldweights only waits on the
            # two weight-sign ops (cg0/cg1) of its own tap group.
            w_s = persist.tile([128, CG, 9, CG, 128], F8)
            stats = persist.tile([128, CG, NLOC * NRT, 6], F32)
            wk0 = persist.tile([128, C * 9], F32)
            wk1 = persist.tile([128, C * 9], F32)
            wks = [wk0, wk1]
            BF16 = mybir.dt.bfloat16
            wk16_0 = persist.tile([128, C * 9], BF16)
            wk16_1 = persist.tile([128, C * 9], BF16)
            wk16s = [wk16_0, wk16_1]

            identity = consts.tile([128, 128], F32)
            make_identity(nc, identity)
            identity16 = consts.tile([128, 128], mybir.dt.bfloat16)
            nc.vector.tensor_copy(out=identity16, in_=identity)
            g_sb = consts.tile([128, CG], F32)
            b_sb = consts.tile([128, CG], F32)
            alpha_sum = consts.tile([128, CG], F32)
            alpha = consts.tile([128, CG], F32)
            alpha2 = consts.tile([128, CG], F32)
            ga = consts.tile([128, CG], F32)
            # Per-kg stats scalars live in SEPARATE tiles: shared tiles with
            # adjacent per-kg slices pick up false cross-kg dependencies from
            # conservative access tracking (a kg1 write ended up waiting on a
            # later kg0 DMA completion), serializing the two stats pipelines.
            scale_t = [consts.tile([128, 1], F32, name=f"scale{k}") for k in range(CG)]
            shift_t = [consts.tile([128, 1], F32, name=f"shift{k}") for k in range(CG)]
            me_t = [consts.tile([128, 2], F32, name=f"me{k}") for k in range(CG)]
            var_t = [consts.tile([128, 1], F32, name=f"var{k}") for k in range(CG)]
            t0_t = [consts.tile([128, 1], F32, name=f"t0_{k}") for k in range(CG)]
            cc_stage_t = [consts.tile([128, 2], F32, name=f"ccs{k}") for k in range(CG)]
            gsum_t = [consts.tile([128, 2], F32, name=f"gsum{k}") for k in range(CG)]
            gath_t = [consts.tile([128, NCORES, 2], F32, name=f"gath{k}") for k in range(CG)]
            dummy = consts.tile([128, 1], F32)
            eps_sb = consts.tile([128, 1], F32)
            nc.vector.memset(eps_sb, EPS)
            # Dummy Sqrt up front: Bacc's table-load inserter then starts on
            # a set containing Sqrt+Sign+Copy+Identity, so no mid-stream
            # ACT table reload ever happens.
            nc.scalar.activation(
                out=dummy, in_=eps_sb,
                func=mybir.ActivationFunctionType.Sqrt,
            )

            # pad-zero memsets for a_s (gpsimd ring; disjoint from interiors).
            # cols 58-59 are dead pitch bytes but taps at dx=2 read them, so
            # zero them too (fp8 garbage would put NaNs in the psum pad cols).
            for n in range(NLOC):
                nc.gpsimd.memset(a_s[:, n, 0, :, :], 0.0)
                nc.gpsimd.memset(a_s[:, n, HP - 1, :, :], 0.0)
                nc.gpsimd.memset(a_s[:, n, :, :, 0:1], 0.0)
                nc.gpsimd.memset(a_s[:, n, :, :, 57:60], 0.0)

            # ---------------- helpers ----------------
            def load_wk(kg):
                nc.sync.dma_start(
                    out=wks[kg],
                    in_=w_d[kg * 128 : (kg + 1) * 128].rearrange(
                        "k c r s -> k (c r s)"
                    ),
                )

            def weight_prep(kg):
                """bf16 cast (DVE), PE transpose, ACT Sign into fp8 w_s.
                bf16 halves the PE transpose cost, which matters in the
                cold-clock window before the convs ramp the PE up."""
                nc.vector.tensor_copy(out=wk16s[kg], in_=wks[kg])
                wk_r = wk16s[kg][:].rearrange("p (c o) -> p c o", o=9)
                for off0 in range(0, 9, 3):
                    for cg in range(CG):
                        pst = psum_t.tile(
                            [128, 3 * 128], mybir.dt.bfloat16,
                            name=f"pst{kg}_{cg}_{off0}", tag="pst",
                        )
                        for j in range(3):
                            nc.tensor.transpose(
                                pst[:, j * 128 : (j + 1) * 128],
                                wk_r[:, cg * 128 : (cg + 1) * 128, off0 + j],
                                identity16,
                            )
                        nc.scalar.activation(
                            out=w_s[:, kg, off0 : off0 + 3, cg, :],
                            in_=pst[:].rearrange("p (j k) -> p j k", k=128),
                            func=mybir.ActivationFunctionType.Sign,
                        )

            def alpha_reduce(kg):
                nc.vector.tensor_reduce(
                    out=alpha_sum[:, kg : kg + 1],
                    in_=wk16s[kg],
                    axis=mybir.AxisListType.X,
                    op=mybir.AluOpType.add,
                    apply_absolute_value=True,
                )

            def alpha_finish():
                nc.vector.tensor_scalar_mul(alpha, alpha_sum, 1.0 / W_RED)
                nc.vector.tensor_mul(alpha2, alpha, alpha)
                nc.vector.tensor_mul(ga, g_sb, alpha)

            xsths = {}

            def dma_half(n, cg, h):
                """Half-image x load into a stage tile (no a_s writes here,
                so prefetching never creates false deps on conv tiles)."""
                xsth = xstage.tile(
                    [128, HR, W], F32, name=f"xsth{cg}_{n}_{h}",
                    tag="stage", bufs=6,
                )
                xsths[(n, cg, h)] = xsth
                nc.sync.dma_start(
                    out=xsth,
                    in_=x_d[n, cg * 128 : (cg + 1) * 128,
                            h * HR : (h + 1) * HR, :],
                )

            def sign_half(n, cg, h):
                """ACT signs a loaded half into a_s; Pool/DVE cast to f16.
                Emitted just-in-time: any conv tile emitted later RAW-waits
                on it under interval tracking, so it must directly precede
                the first tile that truly needs it."""
                xsth = xsths[(n, cg, h)]
                nc.scalar.activation(
                    out=a_s[:, n, 1 + h * HR : 1 + (h + 1) * HR, cg, 1 : W + 1],
                    in_=xsth,
                    func=mybir.ActivationFunctionType.Sign,
                )
                x16_half = x16[:, cg, n, h * HH : (h + 1) * HH]
                # f32->f16 residual cast on DVE: on HW the gpsimd CAST runs
                # ~5.5us per half (0.24 roofline) and back-pressures the x
                # stage ring; DVE does it in ~0.9us via its 2x copy mode.
                nc.vector.tensor_copy(
                    out=x16_half, in_=xsth[:].rearrange("p h w -> p (h w)")
                )

            # evac engine schedule: 'A'=ACT, 'D'=DVE, 'P'=Pool.  During the
            # x-paced load phase ACT is sign-heavy and DVE carries bn_stats,
            # so Pool takes most evacuations; the post-load leftover block
            # spreads them evenly.
            evac_sched = ["P"]

            def conv_tile(kg, n, rt):
                ps = psum_pool.tile(
                    [128, FT], F32, name=f"ps{kg}_{n}_{rt}", tag="ps"
                )
                for off in range(9):
                    dy, dx = off // 3, off % 3
                    r0 = rt * RT + dy
                    nc.tensor.matmul(
                        ps,
                        w_s[:, kg, off, :, :],
                        a_s[:, n, r0 : r0 + RT, :, dx : dx + WP].rearrange(
                            "p r c w -> p c r w"
                        ),
                        start=(off == 0),
                        stop=(off == 8),
                        perf_mode=mybir.MatmulPerfMode.DoubleRow,
                    )
                ps_r = ps[:].rearrange("p (h w) -> p h w", w=WP)
                zt = z16[:, kg, n, rt * RT * W : (rt + 1) * RT * W]
                e = evac_sched.pop(0) if evac_sched else "A"
                if e == "A":
                    nc.scalar.activation(
                        out=zt.rearrange("p (h w) -> p h w", w=W),
                        in_=ps_r[:, :, 0:W],
                        func=mybir.ActivationFunctionType.Copy,
                    )
                else:
                    nc.vector.tensor_copy(
                        out=zt.rearrange("p (h w) -> p h w", w=W),
                        in_=ps_r[:, :, 0:W],
                    )
                nc.vector.bn_stats(out=stats[:, kg, n * NRT + rt, :], in_=zt)

            def local_stats(kg):
                """bn_aggr straight into the gather staging tile: the
                per-core (mean, var) pair is gathered raw; the mean-square
                correction happens post-gather on each core."""
                nc.vector.bn_aggr(out=cc_stage_t[kg], in_=stats[:, kg, :, :])

            cc_outs = {}

            def launch_gather(kg, dma_engine):
                """DMA partials to DRAM and AllGather across the 8 cores.
                Per-kg DRAM pools: adjacent tiles in one pool pick up false
                interval deps (kg1's staging write waited on kg0's collective
                read finishing)."""
                dram = dram0 if kg == 0 else dram1
                cc_in = dram.tile([128, 2], F32, name=f"cc_in{kg}")
                cc_out = dram.tile(
                    [NCORES, 128, 2], F32, addr_space="Shared",
                    name=f"cc_out{kg}",
                )
                cc_outs[kg] = cc_out
                dma_engine.dma_start(out=cc_in, in_=cc_stage_t[kg])
                nc.gpsimd.collective_compute(
                    "AllGather",
                    mybir.AluOpType.bypass,
                    replica_groups=[list(range(NCORES))],
                    ins=[cc_in[:].opt()],
                    outs=[cc_out[:].opt()],
                )

            def finish_stats_parts(kg, dma_engine):
                """Emit-callables for the post-gather scale/shift math,
                grouped <=3 ops so they can be interleaved into another
                stream without overflowing the 4-deep engine wait queues.
                Gathered pairs are (mu_i, var_i); with equal per-core counts
                mu = avg(mu_i), var = avg(var_i) + avg(mu_i^2) - mu^2."""
                cc_out = cc_outs[kg]
                gath = gath_t[kg]
                gsum, me, var, t0 = gsum_t[kg], me_t[kg], var_t[kg], t0_t[kg]
                kgs = slice(kg, kg + 1)

                def p0():
                    dma_engine.dma_start(
                        out=gath, in_=cc_out[:].rearrange("i p j -> p i j")
                    )

                def p1():
                    nc.vector.tensor_reduce(
                        out=gsum,
                        in_=gath[:, :, :].rearrange("p i j -> p j i"),
                        axis=mybir.AxisListType.X,
                        op=mybir.AluOpType.add,
                    )
                    nc.vector.tensor_scalar_mul(me, gsum, 1.0 / NCORES)

                def p2():
                    nc.vector.tensor_mul(
                        gath[:, :, 0], gath[:, :, 0], gath[:, :, 0]
                    )
                    nc.vector.tensor_reduce(
                        out=var,
                        in_=gath[:, :, 0:1].rearrange("p i j -> p j i"),
                        axis=mybir.AxisListType.X,
                        op=mybir.AluOpType.add,
                    )

                def p3():
                    nc.vector.tensor_scalar_mul(var, var, 1.0 / NCORES)
                    nc.vector.tensor_add(var, var, me[:, 1:2])
                    nc.vector.tensor_mul(t0, me[:, 0:1], me[:, 0:1])

                def p4():
                    nc.vector.tensor_sub(var, var, t0)
                    nc.scalar.activation(
                        out=t0, in_=var,
                        func=mybir.ActivationFunctionType.Sqrt,
                        bias=eps_sb, scale=alpha2[:, kgs],
                    )

                def p5():
                    nc.vector.reciprocal(out=t0, in_=t0)
                    nc.vector.tensor_mul(scale_t[kg], ga[:, kgs], t0)

                def p6():
                    nc.vector.tensor_mul(t0, me[:, 0:1], scale_t[kg])
                    nc.vector.tensor_sub(shift_t[kg], b_sb[:, kgs], t0)

                return [p0, p1, p2, p3, p4, p5, p6]

            def finish_stats(kg, dma_engine):
                for p in finish_stats_parts(kg, dma_engine):
                    p()

            def pass2(kg, inject=()):
                inject = list(inject)
                # leading chunks shrink (eighth, eighth, quarter) so the
                # first store launches sooner after the stats land
                Q8 = HH // 4
                chunks = [(0, 0, Q8), (0, Q8, 2 * Q8), (0, 2 * Q8, HH)]
                for n in range(NLOC):
                    s0 = HH if n == 0 else 0
                    for s in range(s0, HW, HH):
                        chunks.append((n, s, s + HH))
                for ci, (n, lo, hi) in enumerate(chunks):
                    if ci >= 1 and inject:
                        inject.pop(0)()
                    sz = hi - lo
                    o_t = xstage.tile(
                        [128, HR, W], F32, name=f"o_t{kg}_{ci}",
                        tag="stage", bufs=6,
                    )
                    o_f = o_t[:].rearrange("p h w -> p (h w)")[:, 0:sz]
                    sl = slice(lo, hi)
                    nc.scalar.activation(
                        out=o_f,
                        in_=z16[:, kg, n, sl],
                        func=mybir.ActivationFunctionType.Identity,
                        scale=scale_t[kg],
                        bias=shift_t[kg],
                    )
                    nc.vector.tensor_add(o_f, o_f, x16[:, kg, n, sl])
                    od_r = o_d[n, kg * 128 : (kg + 1) * 128, :, :].rearrange(
                        "c h w -> c (h w)"
                    )
                    nc.sync.dma_start(out=od_r[:, sl], in_=o_f)

            # ================= emission order =================
            # sync queue: wk0 first (transposes gate everything), first image
            # halves next, wk1 between them, then the full x stream (the
            # stage ring paces it).  Sign ops are emitted just-in-time so
            # conv tiles never pick up false deps on later images' signs.
            dma_half(0, 0, 0)
            sign_half(0, 0, 0)
            load_wk(0)
            weight_prep(0)
            dma_half(0, 1, 0)
            sign_half(0, 1, 0)
            load_wk(1)
            dma_half(0, 0, 1)
            dma_half(0, 1, 1)
            nc.sync.dma_start(out=g_sb, in_=g_d.rearrange("(g p) -> p g", g=CG))
            nc.sync.dma_start(out=b_sb, in_=b_d.rearrange("(g p) -> p g", g=CG))
            alpha_reduce(0)
            # img0's h1 signs go ahead of weight_prep(1)'s ACT ops; the
            # row-interleaved a_s layout means the h0 conv tiles below have
            # no interval overlap with these writes.
            sign_half(0, 0, 1)
            sign_half(0, 1, 1)
            for n in range(1, NLOC):
                for h in range(2):
                    dma_half(n, 0, h)
                    dma_half(n, 1, h)

            # conv tiles: per image, kg0 rows 0-2 (h0-gated), kg1 rows 0-2 as
            # PE filler, kg0 rows 3-6 (h1-gated).  kg1 rows 3-6 run after the
            # whole of kg0 so kg0's stats gather launches earliest.
            # Sign ops are hooked a tile or two in so their WAR dependency
            # (on earlier-emitted conv tile reads that alias the same a_s
            # interval) covers only a couple of in-flight tiles.
            # ALL of kg0 first: on HW the conv stream is PE-instruction
            # bound (~1.9us/tile, slower than the x stream), so kg0's 28
            # tiles finish by ~72us and its stats AllGather — with ~10us of
            # launch latency plus cross-core skew — hides entirely under
            # kg1's 28-tile half.
            for n in range(NLOC):
                evac_sched.extend(list("AADAADA"))
                for rt in range(3):
                    conv_tile(0, n, rt)
                if n == 0:
                    # kg1's weight prep rides here: its PE transposes fill
                    # the slot after img0's first tiles, and its ACT signs
                    # bypass the blocked img0-h1 signs in the wait queue.
                    weight_prep(1)
                    alpha_reduce(1)
                    alpha_finish()
                if n > 0:
                    # h1 signs of this image (img0's were emitted up top)
                    sign_half(n, 0, 1)
                    sign_half(n, 1, 1)
                conv_tile(0, n, 3)
                conv_tile(0, n, 4)
                if n + 1 < NLOC:
                    sign_half(n + 1, 0, 0)
                conv_tile(0, n, 5)
                if n + 1 < NLOC:
                    sign_half(n + 1, 1, 0)
                conv_tile(0, n, 6)

            local_stats(0)
            launch_gather(0, nc.sync)

            for m in range(NLOC):
                evac_sched.extend(list("AADAADA"))
                for rt in range(NRT):
                    conv_tile(1, m, rt)

            local_stats(1)
            launch_gather(1, nc.scalar)

            finish_stats(0, nc.sync)
            # kg1's post-gather math rides inside pass2(0)'s stream in small
            # blocked groups (bypassed via the engines' 4-deep wait queues),
            # so kg1's scale/shift are ready the moment its gather lands.
            pass2(0, inject=finish_stats_parts(1, nc.gpsimd))
            pass2(1)

    nc.compile()
    return nc


_CACHE = {}


def _get_kernel():
    if "nc" not in _CACHE:
        _CACHE["nc"] = _build_kernel()
    return _CACHE["nc"]


def kernel(x, weights, gamma, beta, _trace=False, **_ignored):
    assert x.shape == (N, C, H, W), x.shape
    nc = _get_kernel()
    in_maps = [
        {
            "x": np.ascontiguousarray(x[i * NLOC : (i + 1) * NLOC]),
            "weights": weights,
            "gamma": gamma,
            "beta": beta,
        }
        for i in range(NCORES)
    ]
    res = bass_utils.run_bass_kernel_spmd(
        nc, in_maps, core_ids=list(range(NCORES)), trace=_trace
    )
    out = np.concatenate([res.results[i]["out"] for i in range(NCORES)], axis=0)
    if _trace:
        return out, res
    return out


# revision 38
# speedup vs baseline: 1.0786x; 1.0786x over previous
"""Trainium2 Bass kernel for a ReActNet binary BasicBlock.

Reference computation (per reference.py):
    a   = sign(x)                              # forward of BinaryActivation
    bw  = alpha * sign(w), alpha = mean|w| over (in,kh,kw) per out-channel
    y   = conv3x3(a, bw, stride 1, pad 1)      # NCHW
    out = BN_train(y) * gamma + beta + x       # batch stats over (N,H,W)

Key identities:
  * y = alpha_k * z with z = conv3x3(sign(x), sign(w)) an exact small-integer
    tensor, so the conv runs on the PE array in fp8 DoubleRow mode (+-1 is
    exact in fp8e4) with exact fp32 accumulation.
  * BN(y)*gamma+beta = z*scale_k + bias_k with
        scale_k = gamma_k * alpha_k / sqrt(alpha_k^2 * var_z,k + eps)
        bias_k  = beta_k - mu_z,k * scale_k
    where mu_z/var_z are global batch stats of z. Per-core partial sums are
    exchanged with one small AllGather per 128-channel group; each core
    reduces the 8 gathered partials locally.

Sharding: data-parallel over batch, 4 images per core on 8 cores.

Schedule (per core):
  * x arrives as 16 half-image DMAs; ACT signs each half into the padded
    fp8 buffer, DVE casts it to f16 for the residual (2x copy mode; the
    gpsimd cast measured ~6x slower on HW and back-pressured the ring).
  * a_s is laid out [n][row][cg][64-col] so a conv tile's read interval
    under Tile's interval dependency tracking spans only its own rows —
    just-in-time sign emission then never creates false cross-image or
    cross-half dependencies, and the x stream, signs, and conv tiles
    pipeline at DMA pace.
  * conv tiles run kg0-first (kg1 rows 0-2 interleave as PE load-phase
    work, rows 3-6 follow kg0), so kg0's stats AllGather launches right
    after the last x half and hides under the kg1 leftovers; kg1's
    AllGather queues behind it on the collective cores while kg0's
    output stores stream.
  * per-kg stats scalars live in separate tiles and separate DRAM pools
    (shared tiles/pools picked up false cross-kg deps that serialized
    the two collectives), and kg1's post-gather math is interleaved into
    pass2(0) in <=3-op groups bypassed via the 4-deep engine wait queues.
  * PSUM evacuations split across ACT/DVE per-phase to keep both under
    the x-DMA pace; pass2 streams eighth/quarter-sized leading chunks so
    the first store launches early, then half-image chunks gated only by
    the store DMA.
"""

import numpy as np

try:
    import concourse.bass as bass
except ImportError:  # pragma: no cover
    import sys

    for p in ("/opt/trn_rl_repo", "/root/.axon_site/_ro/trn_rl_repo"):
        sys.path.insert(0, p)
    import concourse.bass as bass

import concourse.tile as tile
from concourse import bacc, bass_utils, mybir
from concourse.masks import make_identity

F32 = mybir.dt.float32
F16 = mybir.dt.float16
F8 = mybir.dt.float8e4

N, C, H, W = 32, 256, 56, 56
NCORES = 8
NLOC = N // NCORES  # images per core
HP, WP = H + 2, W + 2  # zero-padded image
HW = H * W
HR = H // 2  # rows per half-image load
HH = HR * W  # elements per half image
WP2 = 64  # row pitch: 58 cols padded to /16 (DoubleRow stride rule)
RT = 8  # padded rows per PSUM tile
NRT = H // RT  # row tiles per image
FT = RT * WP  # matmul free size (464, incl. 2 pad columns per row)
CG = C // 128  # channel groups of 128
EPS = 1e-5
M_TOTAL = float(N * H * W)  # BN element count per channel
M_LOCAL = float(NLOC * H * W)
W_RED = float(C * 9)  # alpha divisor


def _build_kernel():
    nc = bacc.Bacc(
        "TRN2", target_bir_lowering=False, debug=False, num_devices=NCORES
    )
    x_d = nc.dram_tensor("x", (NLOC, C, H, W), F32, kind="ExternalInput").ap()
    w_d = nc.dram_tensor("weights", (C, C, 3, 3), F32, kind="ExternalInput").ap()
    g_d = nc.dram_tensor("gamma", (C,), F32, kind="ExternalInput").ap()
    b_d = nc.dram_tensor("beta", (C,), F32, kind="ExternalInput").ap()
    o_d = nc.dram_tensor("out", (NLOC, C, H, W), F32, kind="ExternalOutput").ap()

    with tile.TileContext(nc) as tc:
        with (
            tc.tile_pool(name="consts", bufs=1) as consts,
            tc.tile_pool(name="persist", bufs=1) as persist,
            tc.tile_pool(name="xstage", bufs=4) as xstage,
            tc.tile_pool(name="psum", bufs=6, space="PSUM") as psum_pool,
            tc.tile_pool(name="psum_t", bufs=2, space="PSUM") as psum_t,
            tc.tile_pool(name="dram0", bufs=1, space="DRAM") as dram0,
            tc.tile_pool(name="dram1", bufs=1, space="DRAM") as dram1,
        ):
            # ---- persistent SBUF state ----
            # padded sign(x): [n][row][cg][col].  Image outermost and cg
            # interleaved INSIDE the row dim, so a conv tile's read interval
            # spans only rows rt..rt+9 of its own image — h1 signs never
            # falsely conflict with h0 tiles under interval dep tracking.
            a_s = persist.tile([128, NLOC, HP, CG, WP2], F8)
            x16 = persist.tile([128, CG, NLOC, HW], F16)  # x for residual
            z16 = persist.tile([128, CG, NLOC, HW], F16)  # conv output
            # sign(w): [c, kg, off, cg, k].  kg outermost so kg0 reads never
            # alias kg1 writes; off next so each Ldweights only waits on the
            # two weight-sign ops (cg0/cg1) of its own tap group.
            w_s = persist.tile([128, CG, 9, CG, 128], F8)
            stats = persist.tile([128, CG, NLOC * NRT, 6], F32)
            wk0 = persist.tile([128, C * 9], F32)
            wk1 = persist.tile([128, C * 9], F32)
            wks = [wk0, wk1]
            BF16 = mybir.dt.bfloat16
            wk16_0 = persist.tile([128, C * 9], BF16)
            wk16_1 = persist.tile([128, C * 9], BF16)
            wk16s = [wk16_0, wk16_1]

            identity = consts.tile([128, 128], F32)
            make_identity(nc, identity)
            identity16 = consts.tile([128, 128], mybir.dt.bfloat16)
            nc.vector.tensor_copy(out=identity16, in_=identity)
            g_sb = consts.tile([128, CG], F32)
            b_sb = consts.tile([128, CG], F32)
            alpha_sum = consts.tile([128, CG], F32)
            alpha = consts.tile([128, CG], F32)
            alpha2 = consts.tile([128, CG], F32)
            ga = consts.tile([128, CG], F32)
            # Per-kg stats scalars live in SEPARATE tiles: shared tiles with
            # adjacent per-kg slices pick up false cross-kg dependencies from
            # conservative access tracking (a kg1 write ended up waiting on a
            # later kg0 DMA completion), serializing the two stats pipelines.
            scale_t = [consts.tile([128, 1], F32, name=f"scale{k}") for k in range(CG)]
            shift_t = [consts.tile([128, 1], F32, name=f"shift{k}") for k in range(CG)]
            me_t = [consts.tile([128, 2], F32, name=f"me{k}") for k in range(CG)]
            var_t = [consts.tile([128, 1], F32, name=f"var{k}") for k in range(CG)]
            t0_t = [consts.tile([128, 1], F32, name=f"t0_{k}") for k in range(CG)]
            cc_stage_t = [consts.tile([128, 2], F32, name=f"ccs{k}") for k in range(CG)]
            gsum_t = [consts.tile([128, 2], F32, name=f"gsum{k}") for k in range(CG)]
            gath_t = [consts.tile([128, NCORES, 2], F32, name=f"gath{k}") for k in range(CG)]
            dummy = consts.tile([128, 1], F32)
            eps_sb = consts.tile([128, 1], F32)
            nc.vector.memset(eps_sb, EPS)
            # Dummy Sqrt up front: Bacc's table-load inserter then starts on
            # a set containing Sqrt+Sign+Copy+Identity, so no mid-stream
            # ACT table reload ever happens.
            nc.scalar.activation(
                out=dummy, in_=eps_sb,
                func=mybir.ActivationFunctionType.Sqrt,
            )

            # pad-zero memsets for a_s (gpsimd ring; disjoint from interiors).
            # cols 58-59 are dead pitch bytes but taps at dx=2 read them, so
            # zero them too (fp8 garbage would put NaNs in the psum pad cols).
            for n in range(NLOC):
                nc.gpsimd.memset(a_s[:, n, 0, :, :], 0.0)
                nc.gpsimd.memset(a_s[:, n, HP - 1, :, :], 0.0)
                nc.gpsimd.memset(a_s[:, n, :, :, 0:1], 0.0)
                nc.gpsimd.memset(a_s[:, n, :, :, 57:60], 0.0)

            # ---------------- helpers ----------------
            def load_wk(kg):
                nc.sync.dma_start(
                    out=wks[kg],
                    in_=w_d[kg * 128 : (kg + 1) * 128].rearrange(
                        "k c r s -> k (c r s)"
                    ),
                )

            def weight_prep(kg):
                """bf16 cast (DVE), PE transpose, ACT Sign into fp8 w_s.
                bf16 halves the PE transpose cost, which matters in the
                cold-clock window before the convs ramp the PE up."""
                nc.vector.tensor_copy(out=wk16s[kg], in_=wks[kg])
                wk_r = wk16s[kg][:].rearrange("p (c o) -> p c o", o=9)
                for off0 in range(0, 9, 3):
                    for cg in range(CG):
                        pst = psum_t.tile(
                            [128, 3 * 128], mybir.dt.bfloat16,
                            name=f"pst{kg}_{cg}_{off0}", tag="pst",
                        )
                        for j in range(3):
                            nc.tensor.transpose(
                                pst[:, j * 128 : (j + 1) * 128],
                                wk_r[:, cg * 128 : (cg + 1) * 128, off0 + j],
                                identity16,
                            )
                        nc.scalar.activation(
                            out=w_s[:, kg, off0 : off0 + 3, cg, :],
                            in_=pst[:].rearrange("p (j k) -> p j k", k=128),
                            func=mybir.ActivationFunctionType.Sign,
                        )

            def alpha_reduce(kg):
                nc.vector.tensor_reduce(
                    out=alpha_sum[:, kg : kg + 1],
                    in_=wk16s[kg],
                    axis=mybir.AxisListType.X,
                    op=mybir.AluOpType.add,
                    apply_absolute_value=True,
                )

            def alpha_finish():
                nc.vector.tensor_scalar_mul(alpha, alpha_sum, 1.0 / W_RED)
                nc.vector.tensor_mul(alpha2, alpha, alpha)
                nc.vector.tensor_mul(ga, g_sb, alpha)

            xsths = {}

            def dma_half(n, cg, h):
                """Half-image x load into a stage tile (no a_s writes here,
                so prefetching never creates false deps on conv tiles)."""
                xsth = xstage.tile(
                    [128, HR, W], F32, name=f"xsth{cg}_{n}_{h}",
                    tag="stage", bufs=6,
                )
                xsths[(n, cg, h)] = xsth
                nc.sync.dma_start(
                    out=xsth,
                    in_=x_d[n, cg * 128 : (cg + 1) * 128,
                            h * HR : (h + 1) * HR, :],
                )

            def sign_half(n, cg, h):
                """ACT signs a loaded half into a_s; Pool/DVE cast to f16.
                Emitted just-in-time: any conv tile emitted later RAW-waits
                on it under interval tracking, so it must directly precede
                the first tile that truly needs it."""
                xsth = xsths[(n, cg, h)]
                nc.scalar.activation(
                    out=a_s[:, n, 1 + h * HR : 1 + (h + 1) * HR, cg, 1 : W + 1],
                    in_=xsth,
                    func=mybir.ActivationFunctionType.Sign,
                )
                x16_half = x16[:, cg, n, h * HH : (h + 1) * HH]
                # f32->f16 residual cast on DVE: on HW the gpsimd CAST runs
                # ~5.5us per half (0.24 roofline) and back-pressures the x
                # stage ring; DVE does it in ~0.9us via its 2x copy mode.
                nc.vector.tensor_copy(
                    out=x16_half, in_=xsth[:].rearrange("p h w -> p (h w)")
                )

            # evac engine schedule: 'A'=ACT, 'D'=DVE, 'P'=Pool.  During the
            # x-paced load phase ACT is sign-heavy and DVE carries bn_stats,
            # so Pool takes most evacuations; the post-load leftover block
            # spreads them evenly.
            evac_sched = ["P"]

            def conv_tile(kg, n, rt):
                ps = psum_pool.tile(
                    [128, FT], F32, name=f"ps{kg}_{n}_{rt}", tag="ps"
                )
                for off in range(9):
                    dy, dx = off // 3, off % 3
                    r0 = rt * RT + dy
                    nc.tensor.matmul(
                        ps,
                        w_s[:, kg, off, :, :],
                        a_s[:, n, r0 : r0 + RT, :, dx : dx + WP].rearrange(
                            "p r c w -> p c r w"
                        ),
                        start=(off == 0),
                        stop=(off == 8),
                        perf_mode=mybir.MatmulPerfMode.DoubleRow,
                    )
                ps_r = ps[:].rearrange("p (h w) -> p h w", w=WP)
                zt = z16[:, kg, n, rt * RT * W : (rt + 1) * RT * W]
                e = evac_sched.pop(0) if evac_sched else "A"
                if e == "A":
                    nc.scalar.activation(
                        out=zt.rearrange("p (h w) -> p h w", w=W),
                        in_=ps_r[:, :, 0:W],
                        func=mybir.ActivationFunctionType.Copy,
                    )
                else:
                    nc.vector.tensor_copy(
                        out=zt.rearrange("p (h w) -> p h w", w=W),
                        in_=ps_r[:, :, 0:W],
                    )
                nc.vector.bn_stats(out=stats[:, kg, n * NRT + rt, :], in_=zt)

            def local_stats(kg):
                """bn_aggr straight into the gather staging tile: the
                per-core (mean, var) pair is gathered raw; the mean-square
                correction happens post-gather on each core."""
                nc.vector.bn_aggr(out=cc_stage_t[kg], in_=stats[:, kg, :, :])

            cc_outs = {}

            def launch_gather(kg, dma_engine):
                """DMA partials to DRAM and AllGather across the 8 cores.
                Per-kg DRAM pools: adjacent tiles in one pool pick up false
                interval deps (kg1's staging write waited on kg0's collective
                read finishing)."""
                dram = dram0 if kg == 0 else dram1
                cc_in = dram.tile([128, 2], F32, name=f"cc_in{kg}")
                cc_out = dram.tile(
                    [NCORES, 128, 2], F32, addr_space="Shared",
                    name=f"cc_out{kg}",
                )
                cc_outs[kg] = cc_out
                dma_engine.dma_start(out=cc_in, in_=cc_stage_t[kg])
                nc.gpsimd.collective_compute(
                    "AllGather",
                    mybir.AluOpType.bypass,
                    replica_groups=[list(range(NCORES))],
                    ins=[cc_in[:].opt()],
                    outs=[cc_out[:].opt()],
                )

            def finish_stats_parts(kg, dma_engine):
                """Emit-callables for the post-gather scale/shift math,
                grouped <=3 ops so they can be interleaved into another
                stream without overflowing the 4-deep engine wait queues.
                Gathered pairs are (mu_i, var_i); with equal per-core counts
                mu = avg(mu_i), var = avg(var_i) + avg(mu_i^2) - mu^2."""
                cc_out = cc_outs[kg]
                gath = gath_t[kg]
                gsum, me, var, t0 = gsum_t[kg], me_t[kg], var_t[kg], t0_t[kg]
                kgs = slice(kg, kg + 1)

                def p0():
                    dma_engine.dma_start(
                        out=gath, in_=cc_out[:].rearrange("i p j -> p i j")
                    )

                def p1():
                    nc.vector.tensor_reduce(
                        out=gsum,
                        in_=gath[:, :, :].rearrange("p i j -> p j i"),
                        axis=mybir.AxisListType.X,
                        op=mybir.AluOpType.add,
                    )
                    nc.vector.tensor_scalar_mul(me, gsum, 1.0 / NCORES)

                def p2():
                    nc.vector.tensor_mul(
                        gath[:, :, 0], gath[:, :, 0], gath[:, :, 0]
                    )
                    nc.vector.tensor_reduce(
                        out=var,
                        in_=gath[:, :, 0:1].rearrange("p i j -> p j i"),
                        axis=mybir.AxisListType.X,
                        op=mybir.AluOpType.add,
                    )

                def p3():
                    nc.vector.tensor_scalar_mul(var, var, 1.0 / NCORES)
                    nc.vector.tensor_add(var, var, me[:, 1:2])
                    nc.vector.tensor_mul(t0, me[:, 0:1], me[:, 0:1])

                def p4():
                    nc.vector.tensor_sub(var, var, t0)
                    nc.scalar.activation(
                        out=t0, in_=var,
                        func=mybir.ActivationFunctionType.Sqrt,
                        bias=eps_sb, scale=alpha2[:, kgs],
                    )

                def p5():
                    nc.vector.reciprocal(out=t0, in_=t0)
                    nc.vector.tensor_mul(scale_t[kg], ga[:, kgs], t0)

                def p6():
                    nc.vector.tensor_mul(t0, me[:, 0:1], scale_t[kg])
                    nc.vector.tensor_sub(shift_t[kg], b_sb[:, kgs], t0)

                return [p0, p1, p2, p3, p4, p5, p6]

            def finish_stats(kg, dma_engine):
                for p in finish_stats_parts(kg, dma_engine):
                    p()

            def pass2(kg, inject=()):
                inject = list(inject)
                # leading chunks shrink (eighth, eighth, quarter) so the
                # first store launches sooner after the stats land
                Q8 = HH // 4
                chunks = [(0, 0, Q8), (0, Q8, 2 * Q8), (0, 2 * Q8, HH)]
                for n in range(NLOC):
                    s0 = HH if n == 0 else 0
                    for s in range(s0, HW, HH):
                        chunks.append((n, s, s + HH))
                for ci, (n, lo, hi) in enumerate(chunks):
                    if ci >= 1 and inject:
                        inject.pop(0)()
                    sz = hi - lo
                    o_t = xstage.tile(
                        [128, HR, W], F32, name=f"o_t{kg}_{ci}",
                        tag="stage", bufs=6,
                    )
                    o_f = o_t[:].rearrange("p h w -> p (h w)")[:, 0:sz]
                    sl = slice(lo, hi)
                    nc.scalar.activation(
                        out=o_f,
                        in_=z16[:, kg, n, sl],
                        func=mybir.ActivationFunctionType.Identity,
                        scale=scale_t[kg],
                        bias=shift_t[kg],
                    )
                    nc.vector.tensor_add(o_f, o_f, x16[:, kg, n, sl])
                    od_r = o_d[n, kg * 128 : (kg + 1) * 128, :, :].rearrange(
                        "c h w -> c (h w)"
                    )
                    nc.sync.dma_start(out=od_r[:, sl], in_=o_f)

            # ================= emission order =================
            # sync queue: wk0 first (transposes gate everything), first image
            # halves next, wk1 between them, then the full x stream (the
            # stage ring paces it).  Sign ops are emitted just-in-time so
            # conv tiles never pick up false deps on later images' signs.
            dma_half(0, 0, 0)
            sign_half(0, 0, 0)
            load_wk(0)
            weight_prep(0)
            dma_half(0, 1, 0)
            sign_half(0, 1, 0)
            load_wk(1)
            dma_half(0, 0, 1)
            dma_half(0, 1, 1)
            nc.sync.dma_start(out=g_sb, in_=g_d.rearrange("(g p) -> p g", g=CG))
            nc.sync.dma_start(out=b_sb, in_=b_d.rearrange("(g p) -> p g", g=CG))
            alpha_reduce(0)
            # img0's h1 signs go ahead of weight_prep(1)'s ACT ops; the
            # row-interleaved a_s layout means the h0 conv tiles below have
            # no interval overlap with these writes.
            sign_half(0, 0, 1)
            sign_half(0, 1, 1)
            for n in range(1, NLOC):
                for h in range(2):
                    dma_half(n, 0, h)
                    dma_half(n, 1, h)

            # conv tiles: per image, kg0 rows 0-2 (h0-gated), kg1 rows 0-2 as
            # PE filler, kg0 rows 3-6 (h1-gated).  kg1 rows 3-6 run after the
            # whole of kg0 so kg0's stats gather launches earliest.
            # Sign ops are hooked a tile or two in so their WAR dependency
            # (on earlier-emitted conv tile reads that alias the same a_s
            # interval) covers only a couple of in-flight tiles.
            for n in range(NLOC):
                # per-image evac split (ACT is sign-heavy, DVE carries
                # bn_stats + the x16 casts)
                evac_sched.extend(list("AADAADADAA"))
                for rt in range(3):
                    conv_tile(0, n, rt)
                if n == 0:
                    # kg1's weight prep rides here: its PE transposes fill
                    # the slot after img0's first tiles, and its ACT signs
                    # bypass the blocked img0-h1 signs in the wait queue.
                    weight_prep(1)
                    alpha_reduce(1)
                    alpha_finish()
                for rt in range(3):
                    conv_tile(1, n, rt)
                if n > 0:
                    # h1 signs of this image (img0's were emitted up top)
                    sign_half(n, 0, 1)
                    sign_half(n, 1, 1)
                conv_tile(0, n, 3)
                conv_tile(0, n, 4)
                if n + 1 < NLOC:
                    sign_half(n + 1, 0, 0)
                conv_tile(0, n, 5)
                if n + 1 < NLOC:
                    sign_half(n + 1, 1, 0)
                conv_tile(0, n, 6)

            local_stats(0)
            launch_gather(0, nc.sync)

            evac_sched.extend(list("AADAADAADAADAADA"))
            for m in range(NLOC):
                for rt in range(3, NRT):
                    conv_tile(1, m, rt)

            local_stats(1)
            launch_gather(1, nc.scalar)

            finish_stats(0, nc.sync)
            # kg1's post-gather math rides inside pass2(0)'s stream in small
            # blocked groups (bypassed via the engines' 4-deep wait queues),
            # so kg1's scale/shift are ready the moment its gather lands.
            pass2(0, inject=finish_stats_parts(1, nc.gpsimd))
            pass2(1)

    nc.compile()
    return nc


_CACHE = {}


def _get_kernel():
    if "nc" not in _CACHE:
        _CACHE["nc"] = _build_kernel()
    return _CACHE["nc"]


def kernel(x, weights, gamma, beta, _trace=False, **_ignored):
    assert x.shape == (N, C, H, W), x.shape
    nc = _get_kernel()
    in_maps = [
        {
            "x": np.ascontiguousarray(x[i * NLOC : (i + 1) * NLOC]),
            "weights": weights,
            "gamma": gamma,
            "beta": beta,
        }
        for i in range(NCORES)
    ]
    res = bass_utils.run_bass_kernel_spmd(
        nc, in_maps, core_ids=list(range(NCORES)), trace=_trace
    )
    out = np.concatenate([res.results[i]["out"] for i in range(NCORES)], axis=0)
    if _trace:
        return out, res
    return out
